# revision 1
# baseline (speedup 1.0000x reference)
"""SATD-style custom loss on 8 Trainium2 NeuronCores.

Computes sum(|H8 @ (original - pred)|) where H8 is the 8x8 Sylvester
Hadamard matrix applied along dim -2 of [B, C, 8, 8] blocks.

Strategy: pure data parallel over the block-batch dim (8 shards).
Per core:
  - gpsimd DMA loads with inline fp32->bf16 cast (halves on-chip traffic)
  - DVE: diff + 3-stage fast Walsh-Hadamard transform along j
    (butterfly distances 8/16/32 elements within each 64-elem block)
  - ACT: fused Abs + per-partition accumulate (accum_out)
  - final DVE reduce to [128,1] per core; host sums 8x128 partials.
"""

import numpy as np

import concourse.bacc as bacc
import concourse.bass as bass
import concourse.mybir as mybir
from concourse.bass_utils import run_bass_kernel_spmd
from concourse.tile import TileContext

# Problem shape (hardcoded; kernel.py must be self-contained).
N_BLOCKS = 524288
C = 3
N_CORES = 8
ELEMS_PER_CORE = (N_BLOCKS // N_CORES) * C * 64  # 12_582_912
P = 128  # SBUF partitions
F = 8192  # fp32 elems per partition per tile
ROWS = ELEMS_PER_CORE // F  # 1536
T = ROWS // P  # 12 tiles per core
NB = F // 64  # 64 SATD blocks per partition per tile

CAST_ON_DMA = True  # fp32->bf16 during DMA (SWDGE); else cast in the diff op
def _build_program() -> bass.Bass:
    nc = bacc.Bacc("TRN2", debug=False, num_devices=N_CORES)
    dt = mybir.dt

    # Host interleaves o|p per row: x[r] = [o_row_r (F), p_row_r (F)].
    # One DMA per tile -> in-order tile completion, single dep for the diff.
    x_dram = nc.declare_dram_parameter("x", [ROWS, 2 * F], dt.float32, isOutput=False)
    out_dram = nc.declare_dram_parameter("out", [P, 1], dt.float32, isOutput=True)

    in_dt = dt.bfloat16 if CAST_ON_DMA else dt.float32

    with TileContext(nc) as tc:
        with (
            tc.tile_pool(name="io", bufs=3) as io_pool,
            tc.tile_pool(name="work", bufs=2) as work_pool,
            tc.tile_pool(name="acc", bufs=1) as acc_pool,
        ):
            # Tile plan: (row0, col0, width). First and last tiles are split
            # into quarters: small first chunks let DVE start ~7us earlier
            # (it otherwise never recovers the startup lag — DVE and DMA
            # per-tile times are nearly matched); small last chunks shorten
            # the serial drain chain at the end.
            SPLIT = 8
            w_q = F // SPLIT
            plan = (
                [(0, k * w_q, w_q) for k in range(SPLIT)]
                + [(t * P, 0, F) for t in range(1, T - 1)]
                + [((T - 1) * P, k * w_q, w_q) for k in range(SPLIT)]
            )
            acc = acc_pool.tile([P, len(plan)], dt.float32)

            for t, (r0, c0, F_) in enumerate(plan):
                dma_eng = nc.gpsimd if CAST_ON_DMA else nc.sync
                xb = io_pool.tile([P, 2 * F_], in_dt, tag="xb")
                if c0 == 0 and F_ == F:
                    dma_eng.dma_start(out=xb[:], in_=x_dram[r0 : r0 + P, :])
                else:
                    dma_eng.dma_start(
                        out=xb[:, 0:F_], in_=x_dram[r0 : r0 + P, c0 : c0 + F_]
                    )
                    dma_eng.dma_start(
                        out=xb[:, F_ : 2 * F_],
                        in_=x_dram[r0 : r0 + P, F + c0 : F + c0 + F_],
                    )

                # diff of the o-half and p-half. Two work buffers ping-pong
                # through the stages (each stage's input is dead after it),
                # keeping SBUF under budget at F=8192.
                wa = work_pool.tile([P, F_], dt.bfloat16, tag="wa")
                wb = work_pool.tile([P, F_], dt.bfloat16, tag="wb")
                nc.vector.tensor_sub(wa[:], xb[:, 0:F_], xb[:, F_ : 2 * F_])

                # FWHT along j: free offset within a block = j*8 + w.
                # stage 1: combine j-bit0 (element distance 8), wa -> wb
                v0 = wa[:].rearrange("p (b j2 s w) -> p b j2 s w", j2=4, s=2, w=8)
                v1 = wb[:].rearrange("p (b j2 s w) -> p b j2 s w", j2=4, s=2, w=8)
                nc.vector.tensor_add(v1[:, :, :, 0, :], v0[:, :, :, 0, :], v0[:, :, :, 1, :])
                nc.vector.tensor_sub(v1[:, :, :, 1, :], v0[:, :, :, 0, :], v0[:, :, :, 1, :])

                # stage 2: combine j-bit1 (element distance 16), wb -> wa
                w1 = wb[:].rearrange("p (b jh s jl) -> p b jh s jl", jh=2, s=2, jl=16)
                w2 = wa[:].rearrange("p (b jh s jl) -> p b jh s jl", jh=2, s=2, jl=16)
                nc.vector.tensor_add(w2[:, :, :, 0, :], w1[:, :, :, 0, :], w1[:, :, :, 1, :])
                nc.vector.tensor_sub(w2[:, :, :, 1, :], w1[:, :, :, 0, :], w1[:, :, :, 1, :])

                # stage 3: combine j-bit2 (element distance 32), wa -> wb
                x2 = wa[:].rearrange("p (b s jl) -> p b s jl", s=2, jl=32)
                x3 = wb[:].rearrange("p (b s jl) -> p b s jl", s=2, jl=32)
                nc.vector.tensor_add(x3[:, :, 0, :], x2[:, :, 0, :], x2[:, :, 1, :])
                nc.vector.tensor_sub(x3[:, :, 1, :], x2[:, :, 0, :], x2[:, :, 1, :])

                # abs + per-partition running sum for this tile (ACT engine);
                # elementwise out is a dump into wa (dead after stage 3).
                nc.scalar.activation(
                    out=wa[:],
                    in_=wb[:],
                    func=mybir.ActivationFunctionType.Abs,
                    accum_out=acc[:, t : t + 1],
                )

            accsum = acc_pool.tile([P, 1], dt.float32)
            nc.vector.tensor_reduce(
                out=accsum[:],
                in_=acc[:],
                axis=mybir.AxisListType.X,
                op=mybir.AluOpType.add,
            )
            nc.sync.dma_start(out=out_dram[:, :], in_=accsum[:])

    nc.compile()
    return nc


_NC_CACHE: bass.Bass | None = None


def _get_program() -> bass.Bass:
    global _NC_CACHE
    if _NC_CACHE is None:
        _NC_CACHE = _build_program()
    return _NC_CACHE


def run(original: np.ndarray, pred: np.ndarray, trace: bool = False, **kwargs):
    """Shard, run on 8 cores, return (scalar result, BassKernelResults)."""
    o = np.asarray(original, dtype=np.float32).reshape(N_CORES, ROWS, F)
    p = np.asarray(pred, dtype=np.float32).reshape(N_CORES, ROWS, F)
    x = np.concatenate([o, p], axis=2)  # [N_CORES, ROWS, 2F] row-interleaved
    in_maps = [{"x": x[i]} for i in range(N_CORES)]
    nc = _get_program()
    res = run_bass_kernel_spmd(
        nc, in_maps, core_ids=list(range(N_CORES)), trace=trace, **kwargs
    )
    total = np.float64(0.0)
    for r in res.results:
        total += r["out"].astype(np.float64).sum()
    return np.array(total, dtype=np.float32), res


def kernel(original: np.ndarray, pred: np.ndarray) -> np.ndarray:
    out, _ = run(original, pred, trace=False)
    return out



# revision 6
# speedup vs baseline: 2.6311x; 2.6311x over previous
"""SATD-style custom loss on 8 Trainium2 NeuronCores.

Computes sum(|H8 @ (original - pred)|) where H8 is the 8x8 Sylvester
Hadamard matrix applied along dim -2 of [B, C, 8, 8] blocks.

Strategy (v4): pure data parallel over the block-batch dim (8 shards).
  - Host casts inputs to fp8e4 (e4m3): 4x less HBM traffic than fp32;
    quantization costs ~4e-4 rel err on the loss (gate: 2e-2).
  - Host lays each core's data out as [128, 2*98304] fp8: partition
    k = g*8+j (16 block-groups x 8 Hadamard rows), col halves
    [o | p], col n = b*8+w within a group.
  - DMA: three queues in parallel (sync HWDGE, scalar HWDGE, gpsimd
    SWDGE), one slab pair of [128, 4096] fp8 transfers per slab.
  - PE: fp8 DoubleRow matmuls with a block-diagonal [+H8 | -H8]
    stationary compute H8 @ (o - p) straight from the fp8 stream
    (one matmul per 512-col PSUM bank, fp32 accumulate).
  - PSUM drain: pairs of adjacent banks ([128, 1024] fp32, a single
    contiguous PSUM read) are abs-summed by either ACT
    (activation(Abs, accum_out)) or DVE (tensor_reduce add with
    apply_absolute_value), greedily balanced ~885ns vs ~1112ns/op.
    TRN2 allows only one PSUM input per instruction and gpsimd
    cannot touch PSUM at all, so two engines is the ceiling here.
  - Final [128,1] partials DMA'd out; host sums 8x128 in float64.
"""

import numpy as np
import ml_dtypes

import concourse.bacc as bacc
import concourse.bass as bass
import concourse.mybir as mybir
from concourse.bass_utils import run_bass_kernel_spmd
from concourse.tile import TileContext

# Problem shape (hardcoded; kernel.py must be self-contained).
N_BLOCKS = 524288
C = 3
N_CORES = 8
BLOCKS_PER_CORE = (N_BLOCKS // N_CORES) * C  # 196608 8x8 blocks
G = 16  # block-groups stacked on partitions (16 * 8j = 128)
BPG = BLOCKS_PER_CORE // G  # 12288 blocks per group
P = 128
NCOLS = BPG * 8  # 98304 moving columns per half (b*8+w)
BANK_N = 512  # fp32 cols per PSUM bank
TILE_N = 1024  # drain granularity: two adjacent banks
TILES = NCOLS // TILE_N  # 96 double-bank tiles
SLAB_TILES = 4
SLAB_N = TILE_N * SLAB_TILES  # 4096 cols per slab half
SLABS = TILES // SLAB_TILES  # 24

ACT_NS = 885.0  # 1024*0.833 + seq overhead
DVE_NS = 1112.0  # 1024*1.0417 + seq overhead

F8 = ml_dtypes.float8_e4m3  # matches mybir dt.float8e4


def _hadamard8() -> np.ndarray:
    H = np.array([[1.0]], dtype=np.float32)
    while H.shape[0] < 8:
        H = np.block([[H, H], [H, -H]])
    return H


def _build_program() -> bass.Bass:
    nc = bacc.Bacc("TRN2", debug=False, num_devices=N_CORES)
    dt = mybir.dt

    x_dram = nc.declare_dram_parameter("x", [P, 2 * NCOLS], dt.float8e4, isOutput=False)
    w_dram = nc.declare_dram_parameter("w", [P, 256], dt.float8e4, isOutput=False)
    out_dram = nc.declare_dram_parameter("out", [P, 1], dt.float32, isOutput=True)

    with TileContext(nc) as tc:
        with (
            tc.tile_pool(name="io", bufs=4) as io_pool,
            tc.tile_pool(name="wpool", bufs=1) as w_pool,
            tc.tile_pool(name="dump", bufs=4) as dump_pool,
            tc.tile_pool(name="acc", bufs=1) as acc_pool,
            tc.tile_pool(name="psum", bufs=4, space="PSUM") as psum_pool,
        ):
            wt = w_pool.tile([P, 256], dt.float8e4)
            nc.sync.dma_start(out=wt[:], in_=w_dram[:, :])
            lhsT = wt[:].rearrange("p (h m) -> p h m", h=2)

            acc = acc_pool.tile([P, TILES], dt.float32)

            dma_engines = [nc.sync, nc.scalar, nc.gpsimd]
            t_act = 0.0  # greedy engine balance
            t_dve = 0.0
            for s in range(SLABS):
                xb = io_pool.tile([P, 2 * SLAB_N], dt.float8e4, tag="xb")
                eng = dma_engines[s % 3]
                eng.dma_start(
                    out=xb[:, 0:SLAB_N],
                    in_=x_dram[:, s * SLAB_N : (s + 1) * SLAB_N],
                )
                eng.dma_start(
                    out=xb[:, SLAB_N : 2 * SLAB_N],
                    in_=x_dram[:, NCOLS + s * SLAB_N : NCOLS + (s + 1) * SLAB_N],
                )
                xv = xb[:].rearrange("p (h n) -> p h n", h=2)

                for i in range(SLAB_TILES):
                    t = s * SLAB_TILES + i
                    pt = psum_pool.tile([P, TILE_N], dt.float32, tag="pt")
                    for half in range(2):
                        rhs = xv[
                            :, :, i * TILE_N + half * BANK_N : i * TILE_N + (half + 1) * BANK_N
                        ]
                        nc.tensor.matmul(
                            out=pt[:, half * BANK_N : (half + 1) * BANK_N],
                            lhsT=lhsT,
                            rhs=rhs,
                            start=True,
                            stop=True,
                            perf_mode=mybir.MatmulPerfMode.DoubleRow,
                        )
                    if t_act + ACT_NS <= t_dve + DVE_NS:
                        t_act += ACT_NS
                        dump = dump_pool.tile([P, TILE_N], dt.bfloat16, tag="da")
                        nc.scalar.activation(
                            out=dump[:],
                            in_=pt[:],
                            func=mybir.ActivationFunctionType.Abs,
                            accum_out=acc[:, t : t + 1],
                        )
                    else:
                        t_dve += DVE_NS
                        nc.vector.tensor_reduce(
                            out=acc[:, t : t + 1],
                            in_=pt[:],
                            axis=mybir.AxisListType.X,
                            op=mybir.AluOpType.add,
                            apply_absolute_value=True,
                        )

            accsum = acc_pool.tile([P, 1], dt.float32)
            nc.vector.tensor_reduce(
                out=accsum[:],
                in_=acc[:],
                axis=mybir.AxisListType.X,
                op=mybir.AluOpType.add,
            )
            nc.sync.dma_start(out=out_dram[:, :], in_=accsum[:])

    nc.compile()
    return nc


_NC_CACHE: bass.Bass | None = None


def _get_program() -> bass.Bass:
    global _NC_CACHE
    if _NC_CACHE is None:
        _NC_CACHE = _build_program()
    return _NC_CACHE


def _prep_core(a8: np.ndarray, c: int) -> np.ndarray:
    """[N_BLOCKS, C, 8, 8] fp8 -> this core's [128, NCOLS] moving layout."""
    nb = N_BLOCKS // N_CORES
    v = a8[c * nb : (c + 1) * nb].reshape(G, BPG, 8, 8)
    return v.transpose(0, 2, 1, 3).reshape(P, NCOLS)


def _build_weights() -> np.ndarray:
    H = _hadamard8()
    W = np.zeros((P, 256), dtype=np.float32)
    for g in range(G):
        W[g * 8 : (g + 1) * 8, g * 8 : (g + 1) * 8] = H
        W[g * 8 : (g + 1) * 8, 128 + g * 8 : 128 + (g + 1) * 8] = -H
    return W.astype(F8)


def run(original: np.ndarray, pred: np.ndarray, trace: bool = False, **kwargs):
    """Shard, run on 8 cores, return (scalar result, BassKernelResults)."""
    o8 = np.asarray(original, dtype=np.float32).astype(F8)
    p8 = np.asarray(pred, dtype=np.float32).astype(F8)
    w = _build_weights()
    in_maps = []
    for c in range(N_CORES):
        x = np.concatenate([_prep_core(o8, c), _prep_core(p8, c)], axis=1)
        in_maps.append({"x": np.ascontiguousarray(x), "w": w})
    nc = _get_program()
    res = run_bass_kernel_spmd(
        nc, in_maps, core_ids=list(range(N_CORES)), trace=trace, **kwargs
    )
    total = np.float64(0.0)
    for r in res.results:
        total += r["out"].astype(np.float64).sum()
    return np.array(total, dtype=np.float32), res


def kernel(original: np.ndarray, pred: np.ndarray) -> np.ndarray:
    out, _ = run(original, pred, trace=False)
    return out


# revision 10
# speedup vs baseline: 3.0909x; 1.1748x over previous
"""SATD-style custom loss on 8 Trainium2 NeuronCores.

Computes sum(|H8 @ (original - pred)|) where H8 is the 8x8 Sylvester
Hadamard matrix applied along dim -2 of [B, C, 8, 8] blocks.

Strategy (v4): pure data parallel over the block-batch dim (8 shards).
  - Host casts inputs to fp8e4 (e4m3): 4x less HBM traffic than fp32;
    quantization costs ~4e-4 rel err on the loss (gate: 2e-2).
  - Host lays each core's data out as [128, 2*98304] fp8: partition
    k = g*8+j (16 block-groups x 8 Hadamard rows), col halves
    [o | p], col n = b*8+w within a group.
  - DMA: three queues in parallel (sync HWDGE, scalar HWDGE, gpsimd
    SWDGE), one slab pair of [128, 4096] fp8 transfers per slab.
  - PE: fp8 DoubleRow matmuls with a block-diagonal [+H8 | -H8]
    stationary compute H8 @ (o - p) straight from the fp8 stream
    (one matmul per 512-col PSUM bank, fp32 accumulate).
  - PSUM drain: pairs of adjacent banks ([128, 1024] fp32, a single
    contiguous PSUM read) are abs-summed by either ACT
    (activation(Abs, accum_out)) or DVE (tensor_reduce add with
    apply_absolute_value), greedily balanced ~885ns vs ~1112ns/op.
    TRN2 allows only one PSUM input per instruction and gpsimd
    cannot touch PSUM at all, so two engines is the ceiling here.
  - Final [128,1] partials DMA'd out; host sums 8x128 in float64.
"""

import numpy as np
import ml_dtypes

import concourse.bacc as bacc
import concourse.bass as bass
import concourse.mybir as mybir
from concourse.bass_utils import run_bass_kernel_spmd
from concourse.tile import TileContext

# Problem shape (hardcoded; kernel.py must be self-contained).
N_BLOCKS = 524288
C = 3
N_CORES = 8
BLOCKS_PER_CORE = (N_BLOCKS // N_CORES) * C  # 196608 8x8 blocks
G = 16  # block-groups stacked on partitions (16 * 8j = 128)
BPG = BLOCKS_PER_CORE // G  # 12288 blocks per group
P = 128
NCOLS = BPG * 8  # 98304 moving columns per half (b*8+w)
BANK_N = 512  # fp32 cols per PSUM bank
TILE_N = 1024  # drain granularity: two adjacent banks
TILES = NCOLS // TILE_N  # 96 double-bank tiles
SLAB_TILES = 4
SLAB_N = TILE_N * SLAB_TILES  # 4096 cols per slab half
SLABS = TILES // SLAB_TILES  # 24

ACT_NS = 1405.0  # measured: 1113 activation + 292 accumulator-read
DVE_NS = 1200.0  # measured tensor_reduce on [128,1024] from PSUM

F8 = ml_dtypes.float8_e4m3  # matches mybir dt.float8e4


def _hadamard8() -> np.ndarray:
    H = np.array([[1.0]], dtype=np.float32)
    while H.shape[0] < 8:
        H = np.block([[H, H], [H, -H]])
    return H


def _build_program() -> bass.Bass:
    nc = bacc.Bacc("TRN2", debug=False, num_devices=N_CORES)
    dt = mybir.dt

    x_dram = nc.declare_dram_parameter("x", [P, 2 * NCOLS], dt.float8e4, isOutput=False)
    w_dram = nc.declare_dram_parameter("w", [P, 256], dt.float8e4, isOutput=False)
    out_dram = nc.declare_dram_parameter("out", [P, 1], dt.float32, isOutput=True)

    with TileContext(nc) as tc:
        with (
            tc.tile_pool(name="io", bufs=4) as io_pool,
            tc.tile_pool(name="wpool", bufs=1) as w_pool,
            tc.tile_pool(name="dump", bufs=4) as dump_pool,
            tc.tile_pool(name="acc", bufs=1) as acc_pool,
            tc.tile_pool(name="psum", bufs=4, space="PSUM") as psum_pool,
        ):
            wt = w_pool.tile([P, 256], dt.float8e4)
            nc.sync.dma_start(out=wt[:], in_=w_dram[:, :])
            lhsT = wt[:].rearrange("p (h m) -> p h m", h=2)

            acc = acc_pool.tile([P, TILES], dt.float32)

            # ACT engine does zero DMA triggering (it is drain-bound);
            # sync HWDGE carries 2/3 of the stream, gpsimd SWDGE 1/3.
            dma_engines = [nc.sync, nc.sync, nc.gpsimd]
            t_act = 0.0  # greedy engine balance
            t_dve = 0.0
            for s in range(SLABS):
                xb = io_pool.tile([P, 2 * SLAB_N], dt.float8e4, tag="xb")
                eng = dma_engines[s % 3]
                eng.dma_start(
                    out=xb[:, 0:SLAB_N],
                    in_=x_dram[:, s * SLAB_N : (s + 1) * SLAB_N],
                )
                eng.dma_start(
                    out=xb[:, SLAB_N : 2 * SLAB_N],
                    in_=x_dram[:, NCOLS + s * SLAB_N : NCOLS + (s + 1) * SLAB_N],
                )
                xv = xb[:].rearrange("p (h n) -> p h n", h=2)

                for i in range(SLAB_TILES):
                    t = s * SLAB_TILES + i
                    pt = psum_pool.tile([P, TILE_N], dt.float32, tag="pt")
                    # PSUM accumulation targets cannot span banks (ISA check),
                    # so two N=512 matmuls fill the [128,1024] drain tile.
                    for half in range(2):
                        rhs = xv[
                            :,
                            :,
                            i * TILE_N + half * BANK_N : i * TILE_N + (half + 1) * BANK_N,
                        ]
                        nc.tensor.matmul(
                            out=pt[:, half * BANK_N : (half + 1) * BANK_N],
                            lhsT=lhsT,
                            rhs=rhs,
                            start=True,
                            stop=True,
                            perf_mode=mybir.MatmulPerfMode.DoubleRow,
                        )
                    if t_act + ACT_NS <= t_dve + DVE_NS:
                        t_act += ACT_NS
                        dump = dump_pool.tile([P, TILE_N], dt.bfloat16, tag="da")
                        nc.scalar.activation(
                            out=dump[:],
                            in_=pt[:],
                            func=mybir.ActivationFunctionType.Abs,
                            accum_out=acc[:, t : t + 1],
                        )
                    else:
                        t_dve += DVE_NS
                        nc.vector.tensor_reduce(
                            out=acc[:, t : t + 1],
                            in_=pt[:],
                            axis=mybir.AxisListType.X,
                            op=mybir.AluOpType.add,
                            apply_absolute_value=True,
                        )

            accsum = acc_pool.tile([P, 1], dt.float32)
            nc.vector.tensor_reduce(
                out=accsum[:],
                in_=acc[:],
                axis=mybir.AxisListType.X,
                op=mybir.AluOpType.add,
            )
            nc.sync.dma_start(out=out_dram[:, :], in_=accsum[:])

    nc.compile()
    return nc


_NC_CACHE: bass.Bass | None = None


def _get_program() -> bass.Bass:
    global _NC_CACHE
    if _NC_CACHE is None:
        _NC_CACHE = _build_program()
    return _NC_CACHE


def _prep_core(a8: np.ndarray, c: int) -> np.ndarray:
    """[N_BLOCKS, C, 8, 8] fp8 -> this core's [128, NCOLS] moving layout."""
    nb = N_BLOCKS // N_CORES
    v = a8[c * nb : (c + 1) * nb].reshape(G, BPG, 8, 8)
    return v.transpose(0, 2, 1, 3).reshape(P, NCOLS)


def _build_weights() -> np.ndarray:
    H = _hadamard8()
    W = np.zeros((P, 256), dtype=np.float32)
    for g in range(G):
        W[g * 8 : (g + 1) * 8, g * 8 : (g + 1) * 8] = H
        W[g * 8 : (g + 1) * 8, 128 + g * 8 : 128 + (g + 1) * 8] = -H
    return W.astype(F8)


def run(original: np.ndarray, pred: np.ndarray, trace: bool = False, **kwargs):
    """Shard, run on 8 cores, return (scalar result, BassKernelResults)."""
    o8 = np.asarray(original, dtype=np.float32).astype(F8)
    p8 = np.asarray(pred, dtype=np.float32).astype(F8)
    w = _build_weights()
    in_maps = []
    for c in range(N_CORES):
        x = np.concatenate([_prep_core(o8, c), _prep_core(p8, c)], axis=1)
        in_maps.append({"x": np.ascontiguousarray(x), "w": w})
    nc = _get_program()
    res = run_bass_kernel_spmd(
        nc, in_maps, core_ids=list(range(N_CORES)), trace=trace, **kwargs
    )
    total = np.float64(0.0)
    for r in res.results:
        total += r["out"].astype(np.float64).sum()
    return np.array(total, dtype=np.float32), res


def kernel(original: np.ndarray, pred: np.ndarray) -> np.ndarray:
    out, _ = run(original, pred, trace=False)
    return out


# revision 12
# speedup vs baseline: 3.3113x; 1.0713x over previous
"""SATD-style custom loss on 8 Trainium2 NeuronCores.

Computes sum(|H8 @ (original - pred)|) where H8 is the 8x8 Sylvester
Hadamard matrix applied along dim -2 of [B, C, 8, 8] blocks.

Strategy (v4): pure data parallel over the block-batch dim (8 shards).
  - Host casts inputs to fp8e4 (e4m3): 4x less HBM traffic than fp32;
    quantization costs ~4e-4 rel err on the loss (gate: 2e-2).
  - Host lays each core's data out as [128, 2*98304] fp8: partition
    k = g*8+j (16 block-groups x 8 Hadamard rows), col halves
    [o | p], col n = b*8+w within a group.
  - DMA: three queues in parallel (sync HWDGE, scalar HWDGE, gpsimd
    SWDGE), one slab pair of [128, 4096] fp8 transfers per slab.
  - PE: fp8 DoubleRow matmuls with a block-diagonal [+H8 | -H8]
    stationary compute H8 @ (o - p) straight from the fp8 stream
    (one matmul per 512-col PSUM bank, fp32 accumulate).
  - PSUM drain: pairs of adjacent banks ([128, 1024] fp32, a single
    contiguous PSUM read) are abs-summed by either ACT
    (activation(Abs, accum_out)) or DVE (tensor_reduce add with
    apply_absolute_value), greedily balanced ~885ns vs ~1112ns/op.
    TRN2 allows only one PSUM input per instruction and gpsimd
    cannot touch PSUM at all, so two engines is the ceiling here.
  - Final [128,1] partials DMA'd out; host sums 8x128 in float64.
"""

import numpy as np
import ml_dtypes

import concourse.bacc as bacc
import concourse.bass as bass
import concourse.mybir as mybir
from concourse.bass_utils import run_bass_kernel_spmd
from concourse.tile import TileContext

# Problem shape (hardcoded; kernel.py must be self-contained).
N_BLOCKS = 524288
C = 3
N_CORES = 8
BLOCKS_PER_CORE = (N_BLOCKS // N_CORES) * C  # 196608 8x8 blocks
G = 16  # block-groups stacked on partitions (16 * 8j = 128)
BPG = BLOCKS_PER_CORE // G  # 12288 blocks per group
P = 128
NCOLS = BPG * 8  # 98304 moving columns per half (b*8+w)
BANK_N = 512  # fp32 cols per PSUM bank
TILE_N = 1024  # drain granularity: two adjacent banks
TILES = NCOLS // TILE_N  # 96 double-bank tiles
SLAB_TILES = 4
SLAB_N = TILE_N * SLAB_TILES  # 4096 cols per slab half
SLABS = TILES // SLAB_TILES  # 24

ACT_NS = 1405.0  # measured: 1113 activation + 292 accumulator-read
DVE_NS = 1200.0  # measured tensor_reduce on [128,1024] from PSUM

F8 = ml_dtypes.float8_e4m3  # matches mybir dt.float8e4


def _hadamard8() -> np.ndarray:
    H = np.array([[1.0]], dtype=np.float32)
    while H.shape[0] < 8:
        H = np.block([[H, H], [H, -H]])
    return H


def _build_program() -> bass.Bass:
    nc = bacc.Bacc("TRN2", debug=False, num_devices=N_CORES)
    dt = mybir.dt

    x_dram = nc.declare_dram_parameter("x", [P, 2 * NCOLS], dt.float8e4, isOutput=False)
    w_dram = nc.declare_dram_parameter("w", [P, 256], dt.float8e4, isOutput=False)
    out_dram = nc.declare_dram_parameter("out", [P, 1], dt.float32, isOutput=True)

    with TileContext(nc) as tc:
        with (
            tc.tile_pool(name="io", bufs=6) as io_pool,
            tc.tile_pool(name="wpool", bufs=1) as w_pool,
            tc.tile_pool(name="dump", bufs=4) as dump_pool,
            tc.tile_pool(name="acc", bufs=1) as acc_pool,
            tc.tile_pool(name="psum", bufs=4, space="PSUM") as psum_pool,
        ):
            wt = w_pool.tile([P, 256], dt.float8e4)
            nc.sync.dma_start(out=wt[:], in_=w_dram[:, :])
            lhsT = wt[:].rearrange("p (h m) -> p h m", h=2)

            acc = acc_pool.tile([P, TILES], dt.float32)

            # ACT engine does zero DMA triggering (it is drain-bound);
            # sync HWDGE carries 2/3 of the stream, gpsimd SWDGE 1/3.
            dma_engines = [nc.sync, nc.sync, nc.gpsimd]
            t_act = 0.0  # greedy engine balance
            t_dve = 0.0
            for s in range(SLABS):
                xb = io_pool.tile([P, 2 * SLAB_N], dt.float8e4, tag="xb")
                eng = dma_engines[s % 3]
                if s == 0:
                    # fine-grained warmup: quarter-chunks across both DMA
                    # paths so the first matmul starts ~4x sooner.
                    q = SLAB_N // 4
                    for k in range(4):
                        qeng = [nc.sync, nc.gpsimd][k % 2]
                        qeng.dma_start(
                            out=xb[:, k * q : (k + 1) * q],
                            in_=x_dram[:, s * SLAB_N + k * q : s * SLAB_N + (k + 1) * q],
                        )
                        qeng.dma_start(
                            out=xb[:, SLAB_N + k * q : SLAB_N + (k + 1) * q],
                            in_=x_dram[
                                :,
                                NCOLS + s * SLAB_N + k * q : NCOLS + s * SLAB_N + (k + 1) * q,
                            ],
                        )
                else:
                    eng.dma_start(
                        out=xb[:, 0:SLAB_N],
                        in_=x_dram[:, s * SLAB_N : (s + 1) * SLAB_N],
                    )
                    eng.dma_start(
                        out=xb[:, SLAB_N : 2 * SLAB_N],
                        in_=x_dram[:, NCOLS + s * SLAB_N : NCOLS + (s + 1) * SLAB_N],
                    )
                xv = xb[:].rearrange("p (h n) -> p h n", h=2)

                for i in range(SLAB_TILES):
                    t = s * SLAB_TILES + i
                    pt = psum_pool.tile([P, TILE_N], dt.float32, tag="pt")
                    # PSUM accumulation targets cannot span banks (ISA check),
                    # so two N=512 matmuls fill the [128,1024] drain tile.
                    for half in range(2):
                        rhs = xv[
                            :,
                            :,
                            i * TILE_N + half * BANK_N : i * TILE_N + (half + 1) * BANK_N,
                        ]
                        nc.tensor.matmul(
                            out=pt[:, half * BANK_N : (half + 1) * BANK_N],
                            lhsT=lhsT,
                            rhs=rhs,
                            start=True,
                            stop=True,
                            perf_mode=mybir.MatmulPerfMode.DoubleRow,
                        )
                    if t_act + ACT_NS <= t_dve + DVE_NS:
                        t_act += ACT_NS
                        dump = dump_pool.tile([P, TILE_N], dt.bfloat16, tag="da")
                        nc.scalar.activation(
                            out=dump[:],
                            in_=pt[:],
                            func=mybir.ActivationFunctionType.Abs,
                            accum_out=acc[:, t : t + 1],
                        )
                    else:
                        t_dve += DVE_NS
                        nc.vector.tensor_reduce(
                            out=acc[:, t : t + 1],
                            in_=pt[:],
                            axis=mybir.AxisListType.X,
                            op=mybir.AluOpType.add,
                            apply_absolute_value=True,
                        )

            accsum = acc_pool.tile([P, 1], dt.float32)
            nc.vector.tensor_reduce(
                out=accsum[:],
                in_=acc[:],
                axis=mybir.AxisListType.X,
                op=mybir.AluOpType.add,
            )
            nc.sync.dma_start(out=out_dram[:, :], in_=accsum[:])

    nc.compile()
    return nc


_NC_CACHE: bass.Bass | None = None


def _get_program() -> bass.Bass:
    global _NC_CACHE
    if _NC_CACHE is None:
        _NC_CACHE = _build_program()
    return _NC_CACHE


def _prep_core(a8: np.ndarray, c: int) -> np.ndarray:
    """[N_BLOCKS, C, 8, 8] fp8 -> this core's [128, NCOLS] moving layout."""
    nb = N_BLOCKS // N_CORES
    v = a8[c * nb : (c + 1) * nb].reshape(G, BPG, 8, 8)
    return v.transpose(0, 2, 1, 3).reshape(P, NCOLS)


def _build_weights() -> np.ndarray:
    H = _hadamard8()
    W = np.zeros((P, 256), dtype=np.float32)
    for g in range(G):
        W[g * 8 : (g + 1) * 8, g * 8 : (g + 1) * 8] = H
        W[g * 8 : (g + 1) * 8, 128 + g * 8 : 128 + (g + 1) * 8] = -H
    return W.astype(F8)


def run(original: np.ndarray, pred: np.ndarray, trace: bool = False, **kwargs):
    """Shard, run on 8 cores, return (scalar result, BassKernelResults)."""
    o8 = np.asarray(original, dtype=np.float32).astype(F8)
    p8 = np.asarray(pred, dtype=np.float32).astype(F8)
    w = _build_weights()
    in_maps = []
    for c in range(N_CORES):
        x = np.concatenate([_prep_core(o8, c), _prep_core(p8, c)], axis=1)
        in_maps.append({"x": np.ascontiguousarray(x), "w": w})
    nc = _get_program()
    res = run_bass_kernel_spmd(
        nc, in_maps, core_ids=list(range(N_CORES)), trace=trace, **kwargs
    )
    total = np.float64(0.0)
    for r in res.results:
        total += r["out"].astype(np.float64).sum()
    return np.array(total, dtype=np.float32), res


def kernel(original: np.ndarray, pred: np.ndarray) -> np.ndarray:
    out, _ = run(original, pred, trace=False)
    return out


# revision 16
# speedup vs baseline: 3.3978x; 1.0261x over previous
"""SATD-style custom loss on 8 Trainium2 NeuronCores.

Computes sum(|H8 @ (original - pred)|) where H8 is the 8x8 Sylvester
Hadamard matrix applied along dim -2 of [B, C, 8, 8] blocks.

Strategy (v4): pure data parallel over the block-batch dim (8 shards).
  - Host casts inputs to fp8e4 (e4m3): 4x less HBM traffic than fp32;
    quantization costs ~4e-4 rel err on the loss (gate: 2e-2).
  - Host lays each core's data out as [128, 2*98304] fp8: partition
    k = g*8+j (16 block-groups x 8 Hadamard rows), col halves
    [o | p], col n = b*8+w within a group.
  - DMA: three queues in parallel (sync HWDGE, scalar HWDGE, gpsimd
    SWDGE), one slab pair of [128, 4096] fp8 transfers per slab.
  - PE: fp8 DoubleRow matmuls with a block-diagonal [+H8 | -H8]
    stationary compute H8 @ (o - p) straight from the fp8 stream
    (one matmul per 512-col PSUM bank, fp32 accumulate).
  - PSUM drain: pairs of adjacent banks ([128, 1024] fp32, a single
    contiguous PSUM read) are abs-summed by either ACT
    (activation(Abs, accum_out)) or DVE (tensor_reduce add with
    apply_absolute_value), greedily balanced ~885ns vs ~1112ns/op.
    TRN2 allows only one PSUM input per instruction and gpsimd
    cannot touch PSUM at all, so two engines is the ceiling here.
  - Final [128,1] partials DMA'd out; host sums 8x128 in float64.
"""

import numpy as np
import ml_dtypes

import concourse.bacc as bacc
import concourse.bass as bass
import concourse.mybir as mybir
from concourse.bass_utils import run_bass_kernel_spmd
from concourse.tile import TileContext

# Problem shape (hardcoded; kernel.py must be self-contained).
N_BLOCKS = 524288
C = 3
N_CORES = 8
BLOCKS_PER_CORE = (N_BLOCKS // N_CORES) * C  # 196608 8x8 blocks
G = 16  # block-groups stacked on partitions (16 * 8j = 128)
BPG = BLOCKS_PER_CORE // G  # 12288 blocks per group
P = 128
NCOLS = BPG * 8  # 98304 moving columns per half (b*8+w)
BANK_N = 512  # fp32 cols per PSUM bank
TILE_N = 1024  # drain granularity: two adjacent banks
TILES = NCOLS // TILE_N  # 96 double-bank tiles
SLAB_TILES = 4
SLAB_N = TILE_N * SLAB_TILES  # 4096 cols per slab half
SLABS = TILES // SLAB_TILES  # 24

ACT_NS = 1314.0  # measured: activation + accumulator-read per [128,1024]
DVE_NS = 1189.0  # measured tensor_reduce on [128,1024] from PSUM

F8 = ml_dtypes.float8_e4m3  # matches mybir dt.float8e4


def _hadamard8() -> np.ndarray:
    H = np.array([[1.0]], dtype=np.float32)
    while H.shape[0] < 8:
        H = np.block([[H, H], [H, -H]])
    return H


def _build_program() -> bass.Bass:
    nc = bacc.Bacc("TRN2", debug=False, num_devices=N_CORES)
    dt = mybir.dt

    x_dram = nc.declare_dram_parameter("x", [P, 2 * NCOLS], dt.float8e4, isOutput=False)
    w_dram = nc.declare_dram_parameter("w", [P, 256], dt.float8e4, isOutput=False)
    out_dram = nc.declare_dram_parameter("out", [P, 1], dt.float32, isOutput=True)

    with TileContext(nc) as tc:
        with (
            tc.tile_pool(name="io", bufs=6) as io_pool,
            tc.tile_pool(name="wpool", bufs=1) as w_pool,
            tc.tile_pool(name="dump", bufs=4) as dump_pool,
            tc.tile_pool(name="acc", bufs=1) as acc_pool,
            tc.tile_pool(name="psum", bufs=4, space="PSUM") as psum_pool,
        ):
            wt = w_pool.tile([P, 256], dt.float8e4)
            nc.sync.dma_start(out=wt[:], in_=w_dram[:, :])
            lhsT = wt[:].rearrange("p (h m) -> p h m", h=2)

            acc = acc_pool.tile([P, TILES], dt.float32)

            # ACT engine does zero DMA triggering (it is drain-bound);
            # sync HWDGE carries 2/3 of the stream, gpsimd SWDGE 1/3.
            dma_engines = [nc.sync, nc.sync, nc.gpsimd]
            t_act = 0.0  # greedy engine balance
            t_dve = 0.0
            # slab plan: 22 full slabs, then 4 half slabs to taper the
            # drain tail; slabs 0-1 DMA in quarter-chunks on both queues
            # so the first matmuls start sooner.
            plan = [(s * SLAB_N, SLAB_N) for s in range(SLABS - 2)]
            base = (SLABS - 2) * SLAB_N
            plan += [(base + k * (SLAB_N // 2), SLAB_N // 2) for k in range(4)]

            t = 0
            for s, (c0, ncols) in enumerate(plan):
                xb = io_pool.tile([P, 2 * ncols], dt.float8e4, tag="xb")
                eng = dma_engines[s % 3]
                if s < 2:
                    q = ncols // 4
                    for k in range(4):
                        qeng = [nc.sync, nc.gpsimd][(s + k) % 2]
                        qeng.dma_start(
                            out=xb[:, k * q : (k + 1) * q],
                            in_=x_dram[:, c0 + k * q : c0 + (k + 1) * q],
                        )
                        qeng.dma_start(
                            out=xb[:, ncols + k * q : ncols + (k + 1) * q],
                            in_=x_dram[:, NCOLS + c0 + k * q : NCOLS + c0 + (k + 1) * q],
                        )
                else:
                    eng.dma_start(
                        out=xb[:, 0:ncols],
                        in_=x_dram[:, c0 : c0 + ncols],
                    )
                    eng.dma_start(
                        out=xb[:, ncols : 2 * ncols],
                        in_=x_dram[:, NCOLS + c0 : NCOLS + c0 + ncols],
                    )
                xv = xb[:].rearrange("p (h n) -> p h n", h=2)

                for i in range(ncols // TILE_N):
                    pt = psum_pool.tile([P, TILE_N], dt.float32, tag="pt")
                    # PSUM accumulation targets cannot span banks (ISA check),
                    # so two N=512 matmuls fill the [128,1024] drain tile.
                    for half in range(2):
                        rhs = xv[
                            :,
                            :,
                            i * TILE_N + half * BANK_N : i * TILE_N + (half + 1) * BANK_N,
                        ]
                        nc.tensor.matmul(
                            out=pt[:, half * BANK_N : (half + 1) * BANK_N],
                            lhsT=lhsT,
                            rhs=rhs,
                            start=True,
                            stop=True,
                            perf_mode=mybir.MatmulPerfMode.DoubleRow,
                        )
                    if t_act + ACT_NS <= t_dve + DVE_NS:
                        t_act += ACT_NS
                        dump = dump_pool.tile([P, TILE_N], dt.bfloat16, tag="da")
                        nc.scalar.activation(
                            out=dump[:],
                            in_=pt[:],
                            func=mybir.ActivationFunctionType.Abs,
                            accum_out=acc[:, t : t + 1],
                        )
                    else:
                        t_dve += DVE_NS
                        nc.vector.tensor_reduce(
                            out=acc[:, t : t + 1],
                            in_=pt[:],
                            axis=mybir.AxisListType.X,
                            op=mybir.AluOpType.add,
                            apply_absolute_value=True,
                        )
                    t += 1
            assert t == TILES

            accsum = acc_pool.tile([P, 1], dt.float32)
            nc.vector.tensor_reduce(
                out=accsum[:],
                in_=acc[:],
                axis=mybir.AxisListType.X,
                op=mybir.AluOpType.add,
            )
            nc.sync.dma_start(out=out_dram[:, :], in_=accsum[:])

    nc.compile()
    return nc


_NC_CACHE: bass.Bass | None = None


def _get_program() -> bass.Bass:
    global _NC_CACHE
    if _NC_CACHE is None:
        _NC_CACHE = _build_program()
    return _NC_CACHE


def _prep_core(a8: np.ndarray, c: int) -> np.ndarray:
    """[N_BLOCKS, C, 8, 8] fp8 -> this core's [128, NCOLS] moving layout."""
    nb = N_BLOCKS // N_CORES
    v = a8[c * nb : (c + 1) * nb].reshape(G, BPG, 8, 8)
    return v.transpose(0, 2, 1, 3).reshape(P, NCOLS)


def _build_weights() -> np.ndarray:
    H = _hadamard8()
    W = np.zeros((P, 256), dtype=np.float32)
    for g in range(G):
        W[g * 8 : (g + 1) * 8, g * 8 : (g + 1) * 8] = H
        W[g * 8 : (g + 1) * 8, 128 + g * 8 : 128 + (g + 1) * 8] = -H
    return W.astype(F8)


def run(original: np.ndarray, pred: np.ndarray, trace: bool = False, **kwargs):
    """Shard, run on 8 cores, return (scalar result, BassKernelResults)."""
    o8 = np.asarray(original, dtype=np.float32).astype(F8)
    p8 = np.asarray(pred, dtype=np.float32).astype(F8)
    w = _build_weights()
    in_maps = []
    for c in range(N_CORES):
        x = np.concatenate([_prep_core(o8, c), _prep_core(p8, c)], axis=1)
        in_maps.append({"x": np.ascontiguousarray(x), "w": w})
    nc = _get_program()
    res = run_bass_kernel_spmd(
        nc, in_maps, core_ids=list(range(N_CORES)), trace=trace, **kwargs
    )
    total = np.float64(0.0)
    for r in res.results:
        total += r["out"].astype(np.float64).sum()
    return np.array(total, dtype=np.float32), res


def kernel(original: np.ndarray, pred: np.ndarray) -> np.ndarray:
    out, _ = run(original, pred, trace=False)
    return out
